# revision 1
# baseline (speedup 1.0000x reference)
"""LocalAttentionDraftLayer TRN2 Bass kernel.

Strategy: sequence-parallel over B*S across 8 cores (each core gets a
contiguous 1024-token chunk of one batch row, plus a 32-token halo of
preceding tokens, zero-padded at sequence start). Attention is strictly
local (window 32, causal), so no collectives are needed: the halo is
materialized host-side.

Everything on-chip is computed in "transposed land" ([feature, token]
layouts) so that every matmul contraction has its operand on partitions
without any transposes, except the attention probabilities P, which are
transposed on the PE (the classic flash-attention transpose).

Matmuls use dt.float32r (full-rate fp32 on the PE at N>=256, ~1e-4
scale-relative rounding); everything else is fp32.

Per core:
  QT[h,q]   = WqT.T @ xT        (scaled by 1/sqrt(H) on PSUM->SBUF copy)
  KT[h,k]   = WkT.T @ xT        (k padded to 1152 for N=256 score tiles)
  V[k,h]    = xT.T @ WvT        (9 chunks of 128 keys)
  per 128-query block b: scores[q, 256k] -> softmax -> P^T via PE
  per 256-query pair: attnT[h,q] += V.T @ P^T
  draftT    = WoT.T @ attnT + xT
  LN stats via ones-matmul partition reduction; rstd broadcast via K=1
  matmul; mean handled as a rank-1 K=1 correction matmul folded into the
  MLP; ln_w folded into W1 host-side, ln_b folded into the gelu bias.
  h1T       = gelu(W1wT.T @ (draftT*rstd) - w1sum*(mu*rstd) + bias1)
  outT      = W2T.T @ h1T + b2 + draftT
Host transposes outT back and stitches the 8 chunks.
"""

import sys

sys.path.insert(0, "/opt/trn_rl_repo")

from contextlib import ExitStack

import numpy as np

import concourse.bacc as bacc
import concourse.tile as tile
from concourse import mybir
from concourse.bass_utils import run_bass_kernel_spmd

B, S, H = 2, 4096, 1024
WIN = 32
N_CORES = 8
SL = S // 4            # 1024 tokens per core
XW = SL + WIN          # 1056 = halo + chunk
KW = SL + 128          # 1152 key-array width (pad so score tiles are N=256)
NB = SL // 128         # 8 query blocks
NP = NB // 2           # 4 query-block pairs

F32 = mybir.dt.float32
F32R = mybir.dt.float32r
AX = mybir.AxisListType.X
OP = mybir.AluOpType
AF = mybir.ActivationFunctionType

_CACHE = {}
DEBUG_TAPS = False


def _build():
    nc = bacc.Bacc("TRN2", target_bir_lowering=False, debug=False,
                   num_devices=N_CORES)

    def din(name, shape, dt=F32R):
        return nc.dram_tensor(name, shape, dt, kind="ExternalInput").ap()

    xT = din("xT", [H, XW])
    wq = din("wq", [H, H])
    wk = din("wk", [H, H])
    wv = din("wv", [H, H])
    wo = din("wo", [H, H])
    w1 = din("w1", [H, 512])
    w2 = din("w2", [512, H])
    cr_d = din("cr", [128, 1409])       # zeros|ones_c|ones_r(row0)|nw1s(row0)
    cf_d = din("cf", [128, 653], F32)   # m0|mR|ident|b1c|b2c|eps
    outT = nc.dram_tensor("outT", [H, SL], F32, kind="ExternalOutput").ap()
    taps = {}
    if DEBUG_TAPS:
        for nm, sh in [("t0_d", [128, 256]), ("t1_d", [128, 256]),
                       ("pn0_d", [128, 256]), ("pn1_d", [128, 256]),
                       ("ptg0_d", [128, 768]),
                       ("qt_d", [128, 8192]), ("kt_d", [128, 8 * KW]),
                       ("vt_d", [128, 9216]), ("at_d", [128, 8192]),
                       ("draft_d", [128, 8192]), ("drs_d", [128, 8192]),
                       ("h1_d", [128, 4096]), ("statr_d", [1, 3072])]:
            taps[nm] = nc.dram_tensor(nm, sh, F32, kind="ExternalOutput").ap()

    with tile.TileContext(nc) as tc, ExitStack() as ctx:
        sb = ctx.enter_context(tc.tile_pool(name="sb", bufs=1))
        sw = ctx.enter_context(tc.tile_pool(name="sw", bufs=3))
        sx = ctx.enter_context(tc.tile_pool(name="sx", bufs=2))
        ps = ctx.enter_context(tc.tile_pool(name="ps", bufs=3))
        ps4 = ctx.enter_context(tc.tile_pool(name="ps4", bufs=3, space="PSUM"))
        ps3 = ctx.enter_context(tc.tile_pool(name="ps3", bufs=4, space="PSUM"))
        ps1 = ctx.enter_context(tc.tile_pool(name="ps1", bufs=1, space="PSUM"))

        # ---- x^T first (critical path), then packed constants ----
        xt = sb.tile([128, 8 * XW], F32R, tag="xt")
        nc.sync.dma_start(xt[:, :].rearrange("p (c w) -> p c w", c=8),
                          xT.rearrange("(c p) w -> p c w", p=128))
        cr = sb.tile([128, 1409], F32R, tag="cr")
        cf = sb.tile([128, 653], F32, tag="cf")
        zero_sb = cr[:, 0:768]
        ones_c = cr[:, 768:769]
        ones_r = cr[0:1, 769:897]
        nw1s_sb = cr[0:1, 897:1409]
        m0_sb = cf[:, 0:256]
        mR_sb = cf[:, 256:512]
        ident_sb = cf[:, 512:640]
        b1c_sb = cf[:, 640:644]
        b2c_sb = cf[:, 644:652]
        eps_t = cf[0:1, 652:653]

        qt = sb.tile([128, 8 * 1024], F32R, tag="qt")
        kt = sb.tile([128, 8 * KW], F32R, tag="kt")
        vt = sb.tile([128, 9 * 1024], F32R, tag="vt")

        def load_quarter(w_dram, i, ncols=256, nkc=8):
            t = sw.tile([128, nkc * ncols], F32R, tag="w")
            nc.sync.dma_start(
                t[:, :].rearrange("p (c j) -> p c j", c=nkc),
                w_dram.rearrange("(c p) h -> p c h", p=128)
                [:, :, i * ncols:(i + 1) * ncols])
            return t

        # ---- Phase 1: QT = (Wq x^T) * 1/sqrt(H), layout [h-chunk][128, q] ----
        for i in range(4):
            wq_t = load_quarter(wq, i)
            for oc in (2 * i, 2 * i + 1):
                for qn in range(2):
                    pp = ps4.tile([128, 512], F32, tag="pp")
                    for kc in range(8):
                        nc.tensor.matmul(
                            pp[:, :],
                            wq_t[:, kc * 256 + (oc % 2) * 128:
                                 kc * 256 + (oc % 2) * 128 + 128],
                            xt[:, kc * XW + WIN + qn * 512:
                               kc * XW + WIN + (qn + 1) * 512],
                            start=(kc == 0), stop=(kc == 7))
                    nc.any.tensor_scalar_mul(
                        qt[:, oc * 1024 + qn * 512:oc * 1024 + (qn + 1) * 512],
                        pp[:, :], 1.0 / 32.0)

        # ---- Phase 2: KT, layout [h-chunk][128, 1152 keys] ----
        KNS = [(0, 384), (384, 384), (768, 288)]
        for i in range(4):
            wk_t = load_quarter(wk, i)
            for oc in (2 * i, 2 * i + 1):
                for (k0, kn) in KNS:
                    pp = ps4.tile([128, 512], F32, tag="pp")
                    for kc in range(8):
                        nc.tensor.matmul(
                            pp[:, 0:kn],
                            wk_t[:, kc * 256 + (oc % 2) * 128:
                                 kc * 256 + (oc % 2) * 128 + 128],
                            xt[:, kc * XW + k0:kc * XW + k0 + kn],
                            start=(kc == 0), stop=(kc == 7))
                    nc.any.tensor_copy(
                        kt[:, oc * KW + k0:oc * KW + k0 + kn], pp[:, 0:kn])

        # ---- Phase 3: V natural [key-chunk][128, h], 9 chunks ----
        for i in range(4):
            wv_t = load_quarter(wv, i)
            for vc in range(9):
                rows = 32 if vc == 8 else 128
                pp = ps4.tile([128, 512], F32, tag="pp")
                for kc in range(8):
                    nc.tensor.matmul(
                        pp[0:rows, 0:256],
                        xt[:, kc * XW + vc * 128:kc * XW + vc * 128 + rows],
                        wv_t[:, kc * 256:(kc + 1) * 256],
                        start=(kc == 0), stop=(kc == 7))
                nc.any.tensor_copy(
                    vt[0:rows, vc * 1024 + i * 256:vc * 1024 + (i + 1) * 256],
                    pp[0:rows, 0:256])

        if DEBUG_TAPS:
            nc.sync.dma_start(taps["qt_d"], qt[:, :].bitcast(F32))
            nc.sync.dma_start(taps["kt_d"], kt[:, :].bitcast(F32))
            nc.sync.dma_start(taps["vt_d"], vt[:, :].bitcast(F32))

        # consts arrive during the projection phases; pad keys before use
        nc.sync.dma_start(cr[:, :], cr_d)
        nc.sync.dma_start(cf[:, :], cf_d)
        for c in range(8):  # zero the key pad columns [1056, 1152)
            nc.vector.tensor_copy(kt[:, c * KW + XW:(c + 1) * KW],
                                  zero_sb[:, 0:KW - XW])

        # ---- Phase 4: local attention -> attnT [h-chunk][128, q] ----
        at = sb.tile([128, 8 * 1024], F32R, tag="xt")  # reuse xt slot
        for p in range(NP):
            ptg = sx.tile([128, 3 * 256], F32R, tag="ptg")
            nc.any.tensor_copy(ptg[:, :], zero_sb)
            for j in range(2):
                b = 2 * p + j
                sc = ps3.tile([128, 512], F32, tag="sc")
                for kc in range(8):
                    nc.tensor.matmul(
                        sc[:, 0:256],
                        qt[:, kc * 1024 + b * 128:kc * 1024 + (b + 1) * 128],
                        kt[:, kc * KW + b * 128:kc * KW + b * 128 + 256],
                        start=(kc == 0), stop=(kc == 7))
                t = sx.tile([128, 256], F32, tag="p")
                nc.vector.tensor_add(t[:, :], sc[:, 0:256],
                                     (m0_sb if b == 0 else mR_sb))
                nmax = sx.tile([128, 1], F32, tag="nm")
                nc.vector.reduce_max(nmax[:, :], t[:, :], axis=AX, negate=True)
                pexp = sx.tile([128, 256], F32, tag="pe")
                rsum = sx.tile([128, 1], F32, tag="rs")
                nc.scalar.activation(pexp[:, :], t[:, :], AF.Exp,
                                     bias=nmax[:, 0:1], scale=1.0,
                                     accum_out=rsum[:, 0:1])
                rcp = sx.tile([128, 1], F32, tag="rc")
                nc.vector.reciprocal(rcp[:, :], rsum[:, :])
                pn = sx.tile([128, 256], F32, tag="pn")
                nc.vector.tensor_scalar_mul(pn[:, :], pexp[:, :], rcp[:, 0:1])
                if DEBUG_TAPS and b < 2:
                    nc.sync.dma_start(taps[f"t{b}_d"], t[:, :])
                    nc.sync.dma_start(taps[f"pn{b}_d"], pn[:, :])
                # P^T pieces into the pair-group [288k x 256q] layout
                pt1 = ps1.tile([128, 512], F32, tag="pt", name="pt1")
                nc.tensor.transpose(pt1[:, 0:128], pn[:, 0:128], ident_sb)
                nc.any.tensor_copy(ptg[:, j * 384:j * 384 + 128],
                                   pt1[:, 0:128])
                pt2 = ps1.tile([128, 512], F32, tag="pt", name="pt2")
                nc.tensor.transpose(pt2[0:32, 0:128], pn[:, 128:160],
                                    ident_sb)
                nc.any.tensor_copy(ptg[0:32, 256 + j * 384:384 + j * 384],
                                   pt2[0:32, 0:128])
            if DEBUG_TAPS and p == 0:
                nc.sync.dma_start(taps["ptg0_d"], ptg[:, :].bitcast(F32))
            for hgr in range(4):
                # one accumulation region per PSUM bank: on HW, start=True
                # clears the whole bank, so groups must not share a bank
                atp = [ps3.tile([128, 256], F32, tag="sc", name=f"atp{hh}")
                       for hh in range(2)]
                for kc3 in range(3):
                    c = 2 * p + kc3
                    rows = 32 if c == 8 else 128
                    for hh in range(2):
                        hc = 2 * hgr + hh
                        nc.tensor.matmul(
                            atp[hh][:, :],
                            vt[0:rows, c * 1024 + hc * 128:
                               c * 1024 + (hc + 1) * 128],
                            ptg[0:rows, kc3 * 256:(kc3 + 1) * 256],
                            start=(kc3 == 0), stop=(kc3 == 2))
                for hh in range(2):
                    hc = 2 * hgr + hh
                    nc.any.tensor_copy(
                        at[:, hc * 1024 + p * 256:hc * 1024 + (p + 1) * 256],
                        atp[hh][:, :])

        if DEBUG_TAPS:
            nc.sync.dma_start(taps["at_d"], at[:, :].bitcast(F32))

        # ---- Phase 5+6: draftT = Wo attnT + xT; LN stats; drs = draft*rstd.
        # qn-outer so the qn=0 stats chain overlaps the qn=1 Wo matmuls.
        draft = sb.tile([128, 8 * 1024], F32R, tag="qt")  # reuse qt slot
        statr = sb.tile([1, 2048], F32R, tag="statr")
        drs = sb.tile([128, 8 * 1024], F32R, tag="kt")  # reuse kt slot
        for qn in range(2):
            s1 = ps3.tile([1, 512], F32, tag="sc", name=f"s1_{qn}")
            s2 = ps3.tile([1, 512], F32, tag="sc", name=f"s2_{qn}")
            for i in range(4):
                wo_t = load_quarter(wo, i)
                for oc in (2 * i, 2 * i + 1):
                    pp = ps4.tile([128, 512], F32, tag="pp")
                    for kc in range(8):
                        nc.tensor.matmul(
                            pp[:, :],
                            wo_t[:, kc * 256 + (oc % 2) * 128:
                                 kc * 256 + (oc % 2) * 128 + 128],
                            at[:, kc * 1024 + qn * 512:kc * 1024 + (qn + 1) * 512],
                            start=(kc == 0), stop=(kc == 7))
                    xr = sx.tile([128, 512], F32R, tag="xr")
                    nc.sync.dma_start(
                        xr[:, :],
                        xT[oc * 128:(oc + 1) * 128,
                           WIN + qn * 512:WIN + (qn + 1) * 512])
                    dsl = draft[:, oc * 1024 + qn * 512:oc * 1024 + (qn + 1) * 512]
                    nc.vector.tensor_add(dsl, pp[:, :], xr[:, :])
                    nc.tensor.matmul(s1[:, :], ones_c, dsl,
                                     start=(oc == 0), stop=(oc == 7))
                    sq = sx.tile([128, 512], F32R, tag="sq")
                    nc.scalar.square(sq[:, :], dsl)
                    nc.tensor.matmul(s2[:, :], ones_c, sq[:, :],
                                     start=(oc == 0), stop=(oc == 7))
            # stats chain for this qn (overlaps next qn's Wo matmuls)
            nc.vector.tensor_scalar_mul(s1[:, :], s1[:, :], 1.0 / H)
            # mu2 shares the rstd slice (consumed before rstd is written)
            mu2 = statr[0:1, qn * 512:(qn + 1) * 512]
            nc.scalar.square(mu2, s1[:, :])
            nc.vector.tensor_scalar_mul(s2[:, :], s2[:, :], 1.0 / H)
            nc.vector.tensor_sub(s2[:, :], s2[:, :], mu2)
            nc.scalar.activation(s2[:, :], s2[:, :], AF.Sqrt, bias=eps_t)
            rstd = statr[0:1, qn * 512:(qn + 1) * 512]
            with nc.allow_low_precision(reason="f32r is bit-identical to f32"):
                nc.vector.reciprocal(rstd, s2[:, :])
            nc.vector.tensor_mul(statr[0:1, 1024 + qn * 512:1024 + (qn + 1) * 512],
                                 s1[:, :], rstd)
            if qn == 0:
                rb = ps1.tile([128, 512], F32, tag="pt", name="rb")
                nc.tensor.matmul(rb[:, :], ones_r, rstd, start=True, stop=True)
                for oc in range(8):
                    sl = slice(oc * 1024, oc * 1024 + 512)
                    nc.vector.tensor_mul(drs[:, sl], draft[:, sl], rb[:, :])

        if DEBUG_TAPS:
            nc.sync.dma_start(taps["draft_d"], draft[:, :].bitcast(F32))
            nc.sync.dma_start(taps["drs_d"], drs[:, :].bitcast(F32))
            nc.sync.dma_start(taps["statr_d"], statr[:, :].bitcast(F32))

        # ---- Phase 7: h1T = gelu(W1w drs + mean-correction + bias1) ----
        # qn=0 groups first; qn=1's rstd broadcast + scaling is emitted after
        # them so the PE stream does not stall on the qn=1 LN stats chain.
        h1 = sb.tile([128, 4 * 1024], F32R, tag="vt")  # reuse vt slot

        def mlp1_group(w1_t, mc, qn):
            pp = ps4.tile([128, 512], F32, tag="pp", name="pp_m1")
            for kc in range(8):
                nc.tensor.matmul(
                    pp[:, :],
                    w1_t[:, kc * 256 + (mc % 2) * 128:
                         kc * 256 + (mc % 2) * 128 + 128],
                    drs[:, kc * 1024 + qn * 512:kc * 1024 + (qn + 1) * 512],
                    start=(kc == 0), stop=False)
            nc.tensor.matmul(
                pp[:, :],
                nw1s_sb[0:1, mc * 128:(mc + 1) * 128],
                statr[0:1, 1024 + qn * 512:1024 + (qn + 1) * 512],
                start=False, stop=True)
            nc.scalar.activation(
                h1[:, mc * 1024 + qn * 512:mc * 1024 + (qn + 1) * 512],
                pp[:, :], AF.Gelu, bias=b1c_sb[:, mc:mc + 1], scale=1.0)

        w1_ts = []
        for i in range(2):
            w1_t = load_quarter(w1, i)
            w1_ts.append(w1_t)
            for mc in (2 * i, 2 * i + 1):
                mlp1_group(w1_t, mc, 0)
        # deferred qn=1 scaling (hidden under the qn=0 MLP1 groups)
        rb1 = ps1.tile([128, 512], F32, tag="pt", name="rb1")
        nc.tensor.matmul(rb1[:, :], ones_r, statr[0:1, 512:1024],
                         start=True, stop=True)
        for oc in range(8):
            sl = slice(oc * 1024 + 512, oc * 1024 + 1024)
            nc.vector.tensor_mul(drs[:, sl], draft[:, sl], rb1[:, :])
        for i in range(2):
            for mc in (2 * i, 2 * i + 1):
                mlp1_group(w1_ts[i], mc, 1)

        if DEBUG_TAPS:
            nc.sync.dma_start(taps["h1_d"], h1[:, :].bitcast(F32))

        # ---- Phase 8: outT = W2 h1 + b2 + draftT ----
        for i in range(2):
            w2_t = sw.tile([128, 4 * 512], F32R, tag="w")
            nc.sync.dma_start(
                w2_t[:, :].rearrange("p (c j) -> p c j", c=4),
                w2.rearrange("(c p) h -> p c h", p=128)
                [:, :, i * 512:(i + 1) * 512])
            for oc in range(4 * i, 4 * i + 4):
                ot = sx.tile([128, 1024], F32, tag="ot")
                for qn in range(2):
                    pp = ps4.tile([128, 512], F32, tag="pp")
                    for mc in range(4):
                        nc.tensor.matmul(
                            pp[:, :],
                            w2_t[:, mc * 512 + (oc % 4) * 128:
                                 mc * 512 + (oc % 4) * 128 + 128],
                            h1[:, mc * 1024 + qn * 512:mc * 1024 + (qn + 1) * 512],
                            start=(mc == 0), stop=(mc == 3))
                    nc.vector.scalar_tensor_tensor(
                        ot[:, qn * 512:(qn + 1) * 512], pp[:, :],
                        b2c_sb[:, oc:oc + 1],
                        draft[:, oc * 1024 + qn * 512:oc * 1024 + (qn + 1) * 512],
                        op0=OP.add, op1=OP.add)
                nc.sync.dma_start(outT[oc * 128:(oc + 1) * 128, :], ot[:, :])

    nc.compile()
    return nc


def _get_nc():
    if "nc" not in _CACHE:
        _CACHE["nc"] = _build()
    return _CACHE["nc"]


def _masks():
    kk = np.arange(256)[None, :]
    p = np.arange(128)[:, None]
    band = (kk - p >= 1) & (kk - p <= WIN)
    mR = np.where(band, 0.0, -1e30).astype(np.float32)
    m_first = np.where(band & (kk >= WIN), 0.0, -1e30).astype(np.float32)
    return m_first, mR


def kernel(hidden_states, Wq, Wk, Wv, Wo, ln_w, ln_b, W1, b1, W2, b2):
    hs = np.ascontiguousarray(np.asarray(hidden_states, np.float32))
    Wq, Wk, Wv, Wo = (np.asarray(a, np.float32) for a in (Wq, Wk, Wv, Wo))
    ln_w, ln_b = np.asarray(ln_w, np.float32), np.asarray(ln_b, np.float32)
    W1, b1 = np.asarray(W1, np.float32), np.asarray(b1, np.float32)
    W2, b2 = np.asarray(W2, np.float32), np.asarray(b2, np.float32)

    nc = _get_nc()
    m_first, mR = _masks()
    w1T = np.ascontiguousarray(W1.T * ln_w[:, None])
    cr = np.zeros((128, 1409), np.float32)
    cr[:, 768] = 1.0
    cr[0, 769:897] = 1.0
    cr[0, 897:1409] = -w1T.sum(0)
    def cf_pack(m0):
        cf = np.zeros((128, 653), np.float32)
        cf[:, 0:256] = m0
        cf[:, 256:512] = mR
        cf[:, 512:640] = np.eye(128, dtype=np.float32)
        cf[:, 640:644] = (b1 + W1 @ ln_b).reshape(4, 128).T
        cf[:, 644:652] = b2.reshape(8, 128).T
        cf[0, 652] = 1e-5
        return cf
    cf_first, cf_rest = cf_pack(m_first), cf_pack(mR)
    shared = {
        "cr": cr,
        "wq": np.ascontiguousarray(Wq.T),
        "wk": np.ascontiguousarray(Wk.T),
        "wv": np.ascontiguousarray(Wv.T),
        "wo": np.ascontiguousarray(Wo.T),
        "w1": w1T,
        "w2": np.ascontiguousarray(W2.T),
    }
    in_maps = []
    for c in range(N_CORES):
        b, ch = divmod(c, 4)
        rows = hs[b, ch * SL:(ch + 1) * SL]
        halo = (np.zeros((WIN, H), np.float32) if ch == 0
                else hs[b, ch * SL - WIN:ch * SL])
        xT = np.ascontiguousarray(np.concatenate([halo, rows], 0).T)
        m = dict(shared)
        m["xT"] = xT
        m["cf"] = cf_first if ch == 0 else cf_rest
        in_maps.append(m)

    res = run_bass_kernel_spmd(nc, in_maps, list(range(N_CORES)))
    _CACHE["res"] = res
    out = np.empty((B, S, H), np.float32)
    for c in range(N_CORES):
        b, ch = divmod(c, 4)
        out[b, ch * SL:(ch + 1) * SL] = res.results[c]["outT"].T
    return out

